# revision 16
# baseline (speedup 1.0000x reference)
"""CWFA_AO kernel for 8x TRN2 NeuronCores (Bass/Tile), v2.

Math: out[n] = alpha^T W_0 ... W_127 Omega with W_t[n] = A x1 a[n,t] x2 o[n,t]
(encoded).  Since the encoders are linear, W is formed directly from RAW
inputs: W[nt,(il)] = Mraw[pq,nt]^T @ Ahat[pq,(il)] where pq = 17x17 = 289
(raw dims + bias) and Ahat = (Wa'(x)Wo')^T A is precomputed on host.  The
289-deep contraction runs as 3 PE chunks (128+128+33) instead of the naive
1024-deep / 8-chunk form -- 2.7x less PE time; fp16 operands (fp32 PSUM
accumulate).

Recurrence: two 64-step chains (forward from alpha, backward from Omega)
meet in the middle.  Each DVE tick processes both chains x 32 trajectories
on 128 lanes (lane = (g, d, n), g = output-half).  Per tick: one
tensor_tensor multiply [128,512] (fp16 W x fp32 state -> fp16 products),
one deep tensor_reduce (contraction 32 -> 1, fp32 accumulate), and 4
independent partition-offset copies rebuilding the replicated fp32 state.
The state stays fp32 end to end: fp16 state rounding feeds multiplicatively
through the 64-step chain and alone blows the 2e-2 error budget on
ill-conditioned trajectories.  Every dependency edge on the serial chain
costs ~95ns (SBUF write-ack + semaphore), so the tick is structured for 3
edges: multiply -> reduce -> {4 parallel copies} -> next multiply.

Data movement: W tiles are formed 4-ticks-per-tile in PSUM ([128=(q,n),1024]),
evicted to SBUF fp16 by the Act engine, then partition-shuffled into the
per-tick lane layout via a DRAM round trip (4 scattered stores + 1 contiguous
load per 4-tick group) -- HWDGE descriptor-generation cost is a fixed ~625ns
per DMA, so the layouts are chosen to need only 5 DMAs per group.

Sharding: data-parallel over N (32 trajectories per core), replicated weights.
"""

import numpy as np

N, T = 256, 128
DRAW = 16
R = 32
NCORES = 8
NL = N // NCORES          # 32 trajectories per core
TH = T // 2               # 64 steps per chain
NT = NL * T               # 4096 M columns per core (2048 fwd + 2048 bwd)
PQ = 289                  # (16+1) x (16+1) raw outer-product dim
CHUNKS = [(0, 128), (128, 128), (256, 33)]
F32 = np.float32
F16 = np.float16

_CACHE = {}


def _build_bass():
    import concourse.bass as bass
    import concourse.bacc as bacc
    import concourse.mybir as mybir
    import concourse.tile as tile

    fp32 = mybir.dt.float32
    fp16 = mybir.dt.float16
    mult = mybir.AluOpType.mult
    add = mybir.AluOpType.add
    AX = mybir.AxisListType.X
    try:
        ACT_COPY = mybir.ActivationFunctionType.Copy
    except AttributeError:
        ACT_COPY = mybir.ActivationFunctionType.Identity

    nc = bacc.Bacc()

    # ---- DRAM I/O ----
    d_arep = [nc.dram_tensor(f"arep{c}", [r, NT], fp16, kind="ExternalInput")
              for c, (_, r) in enumerate(CHUNKS)]
    d_orep = [nc.dram_tensor(f"orep{c}", [r, NT], fp16, kind="ExternalInput")
              for c, (_, r) in enumerate(CHUNKS)]
    d_heads = nc.dram_tensor("heads", [128, 3072], fp16, kind="ExternalInput")
    d_afb = [nc.dram_tensor(f"afb{c}", [r, 2048], fp16, kind="ExternalInput")
             for c, (_, r) in enumerate(CHUNKS)]
    d_state0 = nc.dram_tensor("state0", [128, 32], fp32, kind="ExternalInput")
    d_sfin = nc.dram_tensor("sfin", [64, 32], fp16, kind="ExternalInput")
    d_out = nc.dram_tensor("out", [NL], fp32, kind="ExternalOutput")
    d_stage = nc.dram_tensor("stage", [2048, 2048], fp16, kind="Internal")

    def ap(t, off, dims):
        return bass.AP(t[:].tensor, off, dims)

    def dap(t, off, dims):
        return bass.AP(t[:].tensor, off, dims)

    with tile.TileContext(nc) as tc:
        with (
            tc.tile_pool(name="consts", bufs=1) as cpool,
            tc.tile_pool(name="mraw", bufs=4) as mpool,
            tc.tile_pool(name="wev", bufs=2) as wpool,
            tc.tile_pool(name="stp", bufs=3) as stpool,
            tc.tile_pool(name="tick", bufs=2) as tpool,
            tc.tile_pool(name="fin", bufs=1) as fpool,
            tc.tile_pool(name="psw", bufs=2, space="PSUM") as psw,
            tc.tile_pool(name="psi", bufs=1, space="PSUM") as psi,
        ):
            # ---- constant loads ----
            # loads: one packed DMA carries the group 0-3 slices of all six
            # arep/orep tensors; bulk tails are issued later (see pipeline)
            # so they don't clog HWDGE/DMA ahead of the first stage/gathers.
            heads = cpool.tile([128, 3072], fp16, tag="heads")
            nc.sync.dma_start(heads[:], d_heads[:])
            arep, orep, afb = [], [], []
            for c, (_, r) in enumerate(CHUNKS):
                ar = cpool.tile([r, NT], fp16, name=f"arep{c}", tag=f"arep{c}")
                orr = cpool.tile([r, NT], fp16, name=f"orep{c}", tag=f"orep{c}")
                a = cpool.tile([r, 2048], fp16, name=f"afb{c}", tag=f"afb{c}")
                nc.sync.dma_start(a[:], d_afb[c][:])
                arep.append(ar)
                orep.append(orr)
                afb.append(a)
            sfin = cpool.tile([64, 32], fp16, tag="sfin")
            nc.sync.dma_start(sfin[:], d_sfin[:])

            # persistent state (double buffered): lane (g,d,n) holds the full
            # 32-vector of its (d,n) chain, replicated across g.  Tick 0 reads
            # the uploaded alpha/Omega broadcast.
            stateA = cpool.tile([128, 32], fp32, tag="stateA")
            stateB = cpool.tile([128, 32], fp32, tag="stateB")
            state = [stateA, stateB]
            nc.sync.dma_start(stateA[:], d_state0[:])

            mraw_g = {}       # g -> [3 chunk tiles]

            def mraw_pieces(g):
                base = 128 * g
                pieces = []
                for c, (_, r) in enumerate(CHUNKS):
                    mr = mpool.tile([r, 128], fp16, name=f"mr{c}", tag=f"mr{c}")
                    if g < 4:
                        aap = heads[0:r, 1024 * c + base:1024 * c + base + 128]
                        oap = heads[0:r, 1024 * c + 512 + base:
                                    1024 * c + 512 + base + 128]
                    else:
                        aap = arep[c][:, base:base + 128]
                        oap = orep[c][:, base:base + 128]
                    nc.gpsimd.tensor_tensor(mr[:], aap, oap, mult)
                    pieces.append(mr)
                mraw_g[g] = pieces

            wfb_u = {}
            st4_u = {}
            last_partials = [None]

            def form_group(g):
                u, d = g >> 1, g & 1
                wps = psw.tile([128, 1024], fp32, tag="wps")
                for c in range(3):
                    lhs = mraw_g[g][c]
                    nc.tensor.matmul(wps[:, 0:512], lhs[:],
                                     afb[c][:, 1024 * d:1024 * d + 512],
                                     start=(c == 0), stop=(c == 2))
                    nc.tensor.matmul(wps[:, 512:1024], lhs[:],
                                     afb[c][:, 1024 * d + 512:1024 * d + 1024],
                                     start=(c == 0), stop=(c == 2))
                if d == 0:
                    wfb_u[u] = wpool.tile([128, 2048], fp16, name="wfb", tag="wfb")
                nc.scalar.activation(wfb_u[u][:, 1024 * d:1024 * d + 1024],
                                     wps[:], ACT_COPY)

            def stage_dir(u, d):
                # M columns are n-major (col = 4n + q), so the scattered DRAM
                # write is affine per partition: one DMA per direction.
                # out addr = 65536*(2g+d) + 512*p + j for in part p, col j.
                # d=0 stages from the Act queue right after its eviction (no
                # cross-engine sem); d=1 from SP.
                eng = nc.scalar if d == 0 else nc.sync
                eng.dma_start(
                    dap(d_stage, 262144 * u + 65536 * d,
                        [[512, 128], [131072, 2], [1, 512]]),
                    ap(wfb_u[u], 1024 * d, [[2048, 128], [512, 2], [1, 512]]))

            def shuffle_group(u):
                wfb_u.pop(u)
                st4 = stpool.tile([128, 2048], fp16, tag="st4")
                nc.sync.dma_start(
                    st4[:],
                    dap(d_stage, 262144 * u, [[2048, 128], [1, 2048]]))
                st4_u[u] = st4

            def tick(tau):
                u, q = tau >> 2, tau & 3
                st4 = st4_u[u]
                cur = state[tau & 1]
                nxt = state[(tau + 1) & 1]
                prod = tpool.tile([128, 512], fp16, tag="prod")
                partials = tpool.tile([128, 16], fp32, tag="partials")
                # prod[(g,d,n), y', x] = W[y', x] * v[x]
                nc.vector.tensor_tensor(
                    ap(prod, 0, [[512, 128], [32, 16], [1, 32]]),
                    ap(st4, 512 * q, [[2048, 128], [32, 16], [1, 32]]),
                    ap(cur, 0, [[32, 128], [0, 16], [1, 32]]),
                    mult)
                # single reduce over x (contraction): 512 -> 16 per lane.
                # Chain latency favours one deep reduce over a fold tree:
                # every dependency edge costs ~95ns of write-ack + sem time.
                # partials is fp32: the reduce accumulates at full precision
                # (cost is charged on the input size, so this is free) and
                # the state is rounded to fp16 once per tick by the copies.
                nc.vector.tensor_reduce(
                    ap(partials, 0, [[16, 128], [1, 16]]),
                    ap(prod, 0, [[512, 128], [32, 16], [1, 32]]),
                    AX, add)
                if tau < TH - 1:
                    # rebuild replicated state with 4 independent copies
                    # (parallel set: one dep layer each side)
                    nc.vector.tensor_scalar_mul(nxt[0:64, 0:16],
                                                partials[0:64, :], 1.0)
                    nc.vector.tensor_scalar_mul(nxt[0:64, 16:32],
                                                partials[64:128, :], 1.0)
                    nc.vector.tensor_scalar_mul(nxt[64:128, 0:16],
                                                partials[0:64, :], 1.0)
                    nc.vector.tensor_scalar_mul(nxt[64:128, 16:32],
                                                partials[64:128, :], 1.0)
                else:
                    last_partials[0] = partials
                if q == 3 and u >= 1:
                    st4_u.pop(u - 1, None)

            # ---------------- pipeline ----------------
            mraw_pieces(0)
            mraw_pieces(1)
            for g in range(32):
                if g + 2 < 32:
                    mraw_pieces(g + 2)
                form_group(g)
                u, d = g >> 1, g & 1
                stage_dir(u, d)
                if d:
                    shuffle_group(u)
                    if u == 0:
                        for c in range(3):
                            nc.sync.dma_start(arep[c][:, 512:2048],
                                              d_arep[c][:, 512:2048])
                            nc.sync.dma_start(orep[c][:, 512:2048],
                                              d_orep[c][:, 512:2048])
                    if u == 2:
                        for c in range(3):
                            nc.sync.dma_start(arep[c][:, 2048:NT],
                                              d_arep[c][:, 2048:NT])
                            nc.sync.dma_start(orep[c][:, 2048:NT],
                                              d_orep[c][:, 2048:NT])
                    for tau in range(4 * u, 4 * u + 4):
                        tick(tau)

            # ---- final: out[n] = sum_y v64[n,y] * u64[n,y] ----
            partials = last_partials[0]
            tmp = fpool.tile([64, 32], fp32, tag="tmp")
            junk = fpool.tile([64, 16], fp32, tag="junk")
            res = fpool.tile([64, 1], fp32, tag="res")
            nc.vector.tensor_scalar_mul(tmp[0:32, 0:16], partials[0:32, :], 1.0)
            nc.vector.tensor_scalar_mul(tmp[32:64, 0:16], partials[64:96, :], 1.0)
            nc.vector.tensor_scalar_mul(tmp[0:32, 16:32], partials[32:64, :], 1.0)
            nc.vector.tensor_scalar_mul(tmp[32:64, 16:32], partials[96:128, :], 1.0)
            nc.vector.tensor_tensor(junk[:], tmp[:, 0:16], tmp[:, 16:32], mult)
            nc.vector.tensor_reduce(res[:], junk[:], AX, add)
            resh = fpool.tile([64, 1], fp16, tag="resh")
            nc.vector.tensor_scalar_mul(resh[:], res[:], 1.0)
            fin_ps = psi.tile([32, 1], fp32, tag="fin_ps")
            nc.tensor.matmul(fin_ps[:], sfin[:], resh[:], start=True, stop=True)
            fin_sb = fpool.tile([32, 1], fp32, tag="fin_sb")
            nc.scalar.activation(fin_sb[:], fin_ps[:], ACT_COPY)
            nc.sync.dma_start(d_out[:], fin_sb[0:32, 0:1])

    nc.compile()
    return nc


def _prep_core(actions, obss):
    """actions/obss: [NL, T, 16] one core -> replicated-row chunk uploads.

    araw'/oraw' are [17, NT] (raw dims + ones row, fwd|bwd column halves);
    arep_c[r] = araw'[(off_c + r) // 17], orep_c[r] = oraw'[(off_c + r) % 17]
    so that Mraw[pq] = arep[pq] * orep[pq] elementwise on device."""
    def enc(x):
        # col = 128*(2u+d) + 4*n + q  (t = 4u+q fwd d=0; t = 127-(4u+q) bwd)
        def half(xh):
            v = xh.reshape(NL, TH // 4, 4, DRAW)        # [n, u, q, k]
            return v.transpose(3, 1, 0, 2).reshape(DRAW, TH // 4, 128)
        fr, br = half(x[:, :TH, :]), half(x[:, :TH - 1:-1, :])
        m = np.concatenate([fr[:, :, None, :], br[:, :, None, :]],
                           axis=2).reshape(DRAW, NT)
        return np.concatenate([m, np.ones((1, NT), F32)], axis=0).astype(F16)
    araw, oraw = enc(actions), enc(obss)
    d = {}
    heads = np.zeros((128, 3072), F16)
    for c, (off, r) in enumerate(CHUNKS):
        pq = np.arange(off, off + r)
        d[f"arep{c}"] = np.ascontiguousarray(araw[pq // 17])
        d[f"orep{c}"] = np.ascontiguousarray(oraw[pq % 17])
        heads[0:r, 1024 * c:1024 * c + 512] = d[f"arep{c}"][:, 0:512]
        heads[0:r, 1024 * c + 512:1024 * c + 1024] = d[f"orep{c}"][:, 0:512]
    d["heads"] = heads
    return d


def _consts(Wa, ba, Wo, bo, alpha, A, Omega):
    Wa1 = np.concatenate([Wa, ba[None, :]], axis=0)   # [17, 32]
    Wo1 = np.concatenate([Wo, bo[None, :]], axis=0)   # [17, 32]
    # Ahat[pq, i, l] = sum_jk Wa1[p,j] Wo1[q,k] A[i,j,k,l]
    Ahat = np.einsum('pj,qk,ijkl->pqil', Wa1, Wo1, A,
                     optimize=True).reshape(PQ, R, R)
    # fwd cols (contract over i): Af[pq, 512g + 32*l' + i] = Ahat[pq, i, 16g+l']
    Afl = Ahat.transpose(0, 2, 1)                      # [pq, l, i]
    Af = np.concatenate([Afl[:, 0:16, :].reshape(PQ, 512),
                         Afl[:, 16:32, :].reshape(PQ, 512)], axis=1)
    # bwd cols (contract over l): Ab[pq, 512g + 32*i' + l] = Ahat[pq, 16g+i', l]
    Ab = np.concatenate([Ahat[:, 0:16, :].reshape(PQ, 512),
                         Ahat[:, 16:32, :].reshape(PQ, 512)], axis=1)
    afb_full = np.concatenate([Af, Ab], axis=1)        # [289, 2048]

    cst = {}
    for c, (off, r) in enumerate(CHUNKS):
        cst[f"afb{c}"] = afb_full[off:off + r].astype(F16)
    state0 = np.zeros((128, 32), F32)
    for gp in range(2):
        state0[64 * gp:64 * gp + 32] = alpha
        state0[64 * gp + 32:64 * gp + 64] = Omega[:, 0]
    cst["state0"] = state0.astype(F32)
    sfin = np.zeros((64, 32), F32)
    for gp in range(2):
        for n in range(32):
            sfin[32 * gp + n, n] = 1.0
    cst["sfin"] = sfin.astype(F16)
    return cst


def kernel(actions, obss, Wa, ba, Wo, bo, alpha, A, Omega):
    actions = np.asarray(actions, F32)
    obss = np.asarray(obss, F32)
    Wa = np.asarray(Wa, F32); ba = np.asarray(ba, F32)
    Wo = np.asarray(Wo, F32); bo = np.asarray(bo, F32)
    alpha = np.asarray(alpha, F32)
    A = np.asarray(A, F32)
    Omega = np.asarray(Omega, F32)

    cst = _consts(Wa, ba, Wo, bo, alpha, A, Omega)
    in_maps = []
    for c in range(NCORES):
        reps = _prep_core(actions[NL * c:NL * c + NL],
                          obss[NL * c:NL * c + NL])
        in_maps.append({**reps, **cst})

    if "nc" not in _CACHE:
        _CACHE["nc"] = _build_bass()
    from concourse.bass_utils import run_bass_kernel_spmd
    r = run_bass_kernel_spmd(_CACHE["nc"], in_maps, list(range(NCORES)))
    outs = []
    for c in range(NCORES):
        o = r.results[c]["out"] if isinstance(r.results[c], dict) else r.results[c]
        outs.append(np.asarray(o, F32).reshape(NL))
    return np.concatenate(outs).astype(F32)


# revision 25
# speedup vs baseline: 1.0153x; 1.0153x over previous
"""CWFA_AO kernel for 8x TRN2 NeuronCores (Bass/Tile), v2.

Math: out[n] = alpha^T W_0 ... W_127 Omega with W_t[n] = A x1 a[n,t] x2 o[n,t]
(encoded).  Since the encoders are linear, W is formed directly from RAW
inputs: W[nt,(il)] = Mraw[pq,nt]^T @ Ahat[pq,(il)] where pq = 17x17 = 289
(raw dims + bias) and Ahat = (Wa'(x)Wo')^T A is precomputed on host.  The
289-deep contraction runs as 3 PE chunks (128+128+33) instead of the naive
1024-deep / 8-chunk form -- 2.7x less PE time; fp16 operands (fp32 PSUM
accumulate).

Recurrence: two 64-step chains (forward from alpha, backward from Omega)
meet in the middle.  Each DVE tick processes both chains x 32 trajectories
on 128 lanes (lane = (g, d, n), g = output-half).  Per tick: one
tensor_tensor multiply [128,512] (fp16 W x fp32 state -> fp16 products),
one deep tensor_reduce (contraction 32 -> 1, fp32 accumulate), and 4
independent partition-offset copies rebuilding the replicated fp32 state.
The state stays fp32 end to end: fp16 state rounding feeds multiplicatively
through the 64-step chain and alone blows the 2e-2 error budget on
ill-conditioned trajectories.  Every dependency edge on the serial chain
costs ~95ns (SBUF write-ack + semaphore), so the tick is structured for 3
edges: multiply -> reduce -> {4 parallel copies} -> next multiply.

Data movement: W tiles are formed 4-ticks-per-tile in PSUM ([128=(q,n),1024]),
evicted to SBUF fp16 by the Act engine, then partition-shuffled into the
per-tick lane layout via a DRAM round trip (4 scattered stores + 1 contiguous
load per 4-tick group) -- HWDGE descriptor-generation cost is a fixed ~625ns
per DMA, so the layouts are chosen to need only 5 DMAs per group.

Sharding: data-parallel over N (32 trajectories per core), replicated weights.
"""

import numpy as np

N, T = 256, 128
DRAW = 16
R = 32
NCORES = 8
NL = N // NCORES          # 32 trajectories per core
TH = T // 2               # 64 steps per chain
NT = NL * T               # 4096 M columns per core (2048 fwd + 2048 bwd)
PQ = 289                  # (16+1) x (16+1) raw outer-product dim
CHUNKS = [(0, 128), (128, 128), (256, 33)]
F32 = np.float32
F16 = np.float16

_CACHE = {}


def _build_bass():
    import concourse.bass as bass
    import concourse.bacc as bacc
    import concourse.mybir as mybir
    import concourse.tile as tile

    fp32 = mybir.dt.float32
    fp16 = mybir.dt.float16
    mult = mybir.AluOpType.mult
    add = mybir.AluOpType.add
    AX = mybir.AxisListType.X
    try:
        ACT_COPY = mybir.ActivationFunctionType.Copy
    except AttributeError:
        ACT_COPY = mybir.ActivationFunctionType.Identity

    nc = bacc.Bacc()

    # ---- DRAM I/O ----
    d_arep = [nc.dram_tensor(f"arep{c}", [r, NT], fp16, kind="ExternalInput")
              for c, (_, r) in enumerate(CHUNKS)]
    d_orep = [nc.dram_tensor(f"orep{c}", [r, NT], fp16, kind="ExternalInput")
              for c, (_, r) in enumerate(CHUNKS)]
    d_heads = nc.dram_tensor("heads", [128, 3072], fp16, kind="ExternalInput")
    d_afb = [nc.dram_tensor(f"afb{c}", [r, 2048], fp16, kind="ExternalInput")
             for c, (_, r) in enumerate(CHUNKS)]
    d_state0 = nc.dram_tensor("state0", [128, 32], fp32, kind="ExternalInput")
    d_sfin = nc.dram_tensor("sfin", [64, 32], fp16, kind="ExternalInput")
    d_out = nc.dram_tensor("out", [NL], fp32, kind="ExternalOutput")
    d_stage = nc.dram_tensor("stage", [2048, 2048], fp16, kind="Internal")

    def ap(t, off, dims):
        return bass.AP(t[:].tensor, off, dims)

    def dap(t, off, dims):
        return bass.AP(t[:].tensor, off, dims)

    with tile.TileContext(nc) as tc:
        with (
            tc.tile_pool(name="consts", bufs=1) as cpool,
            tc.tile_pool(name="mraw", bufs=4) as mpool,
            tc.tile_pool(name="wev", bufs=2) as wpool,
            tc.tile_pool(name="stp", bufs=3) as stpool,
            tc.tile_pool(name="tick", bufs=2) as tpool,
            tc.tile_pool(name="fin", bufs=1) as fpool,
            tc.tile_pool(name="psw", bufs=2, space="PSUM") as psw,
            tc.tile_pool(name="psi", bufs=1, space="PSUM") as psi,
        ):
            # ---- constant loads ----
            # loads: state0 goes first -- it is tiny and the PE p-state
            # warm-up below depends only on it.  One packed DMA then carries
            # the group 0-3 slices of all six arep/orep tensors; bulk tails
            # are issued later (see pipeline) so they don't clog HWDGE/DMA
            # ahead of the first stage/gathers.
            stateA = cpool.tile([128, 32], fp32, tag="stateA")
            stateB = cpool.tile([128, 32], fp32, tag="stateB")
            state = [stateA, stateB]
            nc.sync.dma_start(stateA[:], d_state0[:])
            heads = cpool.tile([128, 3072], fp16, tag="heads")
            nc.sync.dma_start(heads[:], d_heads[:])
            arep, orep, afb = [], [], []
            for c, (_, r) in enumerate(CHUNKS):
                ar = cpool.tile([r, NT], fp16, name=f"arep{c}", tag=f"arep{c}")
                orr = cpool.tile([r, NT], fp16, name=f"orep{c}", tag=f"orep{c}")
                a = cpool.tile([r, 2048], fp16, name=f"afb{c}", tag=f"afb{c}")
                nc.sync.dma_start(a[:], d_afb[c][:])
                arep.append(ar)
                orep.append(orr)
                afb.append(a)
            sfin = cpool.tile([64, 32], fp16, tag="sfin")
            nc.sync.dma_start(sfin[:], d_sfin[:])

            # PE p-state warm-up: the PE reaches 2.4GHz only after ~3us of
            # continuous work; without this the first two formation groups
            # run at half clock on the startup critical path.  Scratch
            # matmuls on state0 (the earliest-arriving load) ramp the clock
            # while the heads DMA and first Mraw pieces are still in flight.
            warmA = psi.tile([32, 32], fp32, tag="warmA")
            warmB = psi.tile([32, 32], fp32, tag="warmB")
            for i in range(32):
                w = warmA if i & 1 else warmB
                nc.tensor.matmul(w[:], stateA[:, 0:32], stateA[:, 0:32],
                                 start=True, stop=True, skip_group_check=True)

            mraw_g = {}       # g -> [3 chunk tiles]

            def mraw_pieces(g):
                base = 128 * g
                pieces = []
                for c, (_, r) in enumerate(CHUNKS):
                    mr = mpool.tile([r, 128], fp16, name=f"mr{c}", tag=f"mr{c}")
                    if g < 4:
                        aap = heads[0:r, 1024 * c + base:1024 * c + base + 128]
                        oap = heads[0:r, 1024 * c + 512 + base:
                                    1024 * c + 512 + base + 128]
                    else:
                        aap = arep[c][:, base:base + 128]
                        oap = orep[c][:, base:base + 128]
                    nc.gpsimd.tensor_tensor(mr[:], aap, oap, mult)
                    pieces.append(mr)
                mraw_g[g] = pieces

            wfb_u = {}
            st4_u = {}
            last_partials = [None]

            def form_group(g):
                u, d = g >> 1, g & 1
                wps = psw.tile([128, 1024], fp32, tag="wps")
                for c in range(3):
                    lhs = mraw_g[g][c]
                    nc.tensor.matmul(wps[:, 0:512], lhs[:],
                                     afb[c][:, 1024 * d:1024 * d + 512],
                                     start=(c == 0), stop=(c == 2))
                    nc.tensor.matmul(wps[:, 512:1024], lhs[:],
                                     afb[c][:, 1024 * d + 512:1024 * d + 1024],
                                     start=(c == 0), stop=(c == 2))
                if d == 0:
                    wfb_u[u] = wpool.tile([128, 2048], fp16, name="wfb", tag="wfb")
                nc.scalar.activation(wfb_u[u][:, 1024 * d:1024 * d + 1024],
                                     wps[:], ACT_COPY)

            def stage_dir(u, d):
                # M columns are n-major (col = 4n + q), so the scattered DRAM
                # write is affine per partition: one DMA per direction.
                # out addr = 65536*(2g+d) + 512*p + j for in part p, col j.
                # d=0 stages from the Act queue right after its eviction (no
                # cross-engine sem); d=1 from SP.
                eng = nc.scalar if d == 0 else nc.sync
                eng.dma_start(
                    dap(d_stage, 262144 * u + 65536 * d,
                        [[512, 128], [131072, 2], [1, 512]]),
                    ap(wfb_u[u], 1024 * d, [[2048, 128], [512, 2], [1, 512]]))

            st_q0 = {}

            def shuffle_group(u):
                wfb_u.pop(u)
                if u == 0:
                    # group 0 gathers per-tick: each early tick unblocks on a
                    # 364ns transfer instead of the full 1456ns group gather.
                    for q in range(4):
                        stq = stpool.tile([128, 512], fp16, name=f"stq{q}",
                                          tag=f"stq{q}")
                        nc.sync.dma_start(
                            stq[:],
                            dap(d_stage, 512 * q, [[2048, 128], [1, 512]]))
                        st_q0[q] = stq
                    st4_u[u] = None
                    return
                st4 = stpool.tile([128, 2048], fp16, tag="st4")
                nc.sync.dma_start(
                    st4[:],
                    dap(d_stage, 262144 * u, [[2048, 128], [1, 2048]]))
                st4_u[u] = st4

            def tick(tau):
                u, q = tau >> 2, tau & 3
                if u == 0:
                    # group 0 uses the per-tick gather tiles (pitch 512)
                    st4, pitch, off = st_q0[q], 512, 0
                else:
                    st4, pitch, off = st4_u[u], 2048, 512 * q
                cur = state[tau & 1]
                nxt = state[(tau + 1) & 1]
                prod = tpool.tile([128, 512], fp16, tag="prod")
                partials = tpool.tile([128, 16], fp32, tag="partials")
                # prod[(g,d,n), y', x] = W[y', x] * v[x]
                nc.vector.tensor_tensor(
                    ap(prod, 0, [[512, 128], [32, 16], [1, 32]]),
                    ap(st4, off, [[pitch, 128], [32, 16], [1, 32]]),
                    ap(cur, 0, [[32, 128], [0, 16], [1, 32]]),
                    mult)
                # single reduce over x (contraction): 512 -> 16 per lane.
                # Chain latency favours one deep reduce over a fold tree:
                # every dependency edge costs ~95ns of write-ack + sem time.
                # partials is fp32: the reduce accumulates at full precision
                # (cost is charged on the input size, so this is free) and
                # the state is rounded to fp16 once per tick by the copies.
                nc.vector.tensor_reduce(
                    ap(partials, 0, [[16, 128], [1, 16]]),
                    ap(prod, 0, [[512, 128], [32, 16], [1, 32]]),
                    AX, add)
                if tau < TH - 1:
                    # rebuild replicated state with 4 independent copies
                    # (parallel set: one dep layer each side)
                    nc.vector.tensor_scalar_mul(nxt[0:64, 0:16],
                                                partials[0:64, :], 1.0)
                    nc.vector.tensor_scalar_mul(nxt[0:64, 16:32],
                                                partials[64:128, :], 1.0)
                    nc.vector.tensor_scalar_mul(nxt[64:128, 0:16],
                                                partials[0:64, :], 1.0)
                    nc.vector.tensor_scalar_mul(nxt[64:128, 16:32],
                                                partials[64:128, :], 1.0)
                else:
                    last_partials[0] = partials
                if q == 3 and u >= 1:
                    st4_u.pop(u - 1, None)

            # ---------------- pipeline ----------------
            mraw_pieces(0)
            mraw_pieces(1)
            for g in range(32):
                if g + 2 < 32:
                    mraw_pieces(g + 2)
                form_group(g)
                u, d = g >> 1, g & 1
                stage_dir(u, d)
                if d:
                    shuffle_group(u)
                    if u == 0:
                        for c in range(3):
                            nc.sync.dma_start(arep[c][:, 512:2048],
                                              d_arep[c][:, 512:2048])
                            nc.sync.dma_start(orep[c][:, 512:2048],
                                              d_orep[c][:, 512:2048])
                    if u == 2:
                        for c in range(3):
                            nc.sync.dma_start(arep[c][:, 2048:NT],
                                              d_arep[c][:, 2048:NT])
                            nc.sync.dma_start(orep[c][:, 2048:NT],
                                              d_orep[c][:, 2048:NT])
                    for tau in range(4 * u, 4 * u + 4):
                        tick(tau)

            # ---- final: out[n] = sum_y v64[n,y] * u64[n,y] ----
            partials = last_partials[0]
            tmp = fpool.tile([64, 32], fp32, tag="tmp")
            junk = fpool.tile([64, 16], fp32, tag="junk")
            res = fpool.tile([64, 1], fp32, tag="res")
            nc.vector.tensor_scalar_mul(tmp[0:32, 0:16], partials[0:32, :], 1.0)
            nc.vector.tensor_scalar_mul(tmp[32:64, 0:16], partials[64:96, :], 1.0)
            nc.vector.tensor_scalar_mul(tmp[0:32, 16:32], partials[32:64, :], 1.0)
            nc.vector.tensor_scalar_mul(tmp[32:64, 16:32], partials[96:128, :], 1.0)
            nc.vector.tensor_tensor(junk[:], tmp[:, 0:16], tmp[:, 16:32], mult)
            nc.vector.tensor_reduce(res[:], junk[:], AX, add)
            resh = fpool.tile([64, 1], fp16, tag="resh")
            nc.vector.tensor_scalar_mul(resh[:], res[:], 1.0)
            fin_ps = psi.tile([32, 1], fp32, tag="fin_ps")
            nc.tensor.matmul(fin_ps[:], sfin[:], resh[:], start=True, stop=True)
            fin_sb = fpool.tile([32, 1], fp32, tag="fin_sb")
            nc.scalar.activation(fin_sb[:], fin_ps[:], ACT_COPY)
            nc.sync.dma_start(d_out[:], fin_sb[0:32, 0:1])

    nc.compile()
    return nc


def _prep_core(actions, obss):
    """actions/obss: [NL, T, 16] one core -> replicated-row chunk uploads.

    araw'/oraw' are [17, NT] (raw dims + ones row, fwd|bwd column halves);
    arep_c[r] = araw'[(off_c + r) // 17], orep_c[r] = oraw'[(off_c + r) % 17]
    so that Mraw[pq] = arep[pq] * orep[pq] elementwise on device."""
    def enc(x):
        # col = 128*(2u+d) + 4*n + q  (t = 4u+q fwd d=0; t = 127-(4u+q) bwd)
        def half(xh):
            v = xh.reshape(NL, TH // 4, 4, DRAW)        # [n, u, q, k]
            return v.transpose(3, 1, 0, 2).reshape(DRAW, TH // 4, 128)
        fr, br = half(x[:, :TH, :]), half(x[:, :TH - 1:-1, :])
        m = np.concatenate([fr[:, :, None, :], br[:, :, None, :]],
                           axis=2).reshape(DRAW, NT)
        return np.concatenate([m, np.ones((1, NT), F32)], axis=0).astype(F16)
    araw, oraw = enc(actions), enc(obss)
    d = {}
    heads = np.zeros((128, 3072), F16)
    for c, (off, r) in enumerate(CHUNKS):
        pq = np.arange(off, off + r)
        d[f"arep{c}"] = np.ascontiguousarray(araw[pq // 17])
        d[f"orep{c}"] = np.ascontiguousarray(oraw[pq % 17])
        heads[0:r, 1024 * c:1024 * c + 512] = d[f"arep{c}"][:, 0:512]
        heads[0:r, 1024 * c + 512:1024 * c + 1024] = d[f"orep{c}"][:, 0:512]
    d["heads"] = heads
    return d


def _consts(Wa, ba, Wo, bo, alpha, A, Omega):
    Wa1 = np.concatenate([Wa, ba[None, :]], axis=0)   # [17, 32]
    Wo1 = np.concatenate([Wo, bo[None, :]], axis=0)   # [17, 32]
    # Ahat[pq, i, l] = sum_jk Wa1[p,j] Wo1[q,k] A[i,j,k,l]
    Ahat = np.einsum('pj,qk,ijkl->pqil', Wa1, Wo1, A,
                     optimize=True).reshape(PQ, R, R)
    # fwd cols (contract over i): Af[pq, 512g + 32*l' + i] = Ahat[pq, i, 16g+l']
    Afl = Ahat.transpose(0, 2, 1)                      # [pq, l, i]
    Af = np.concatenate([Afl[:, 0:16, :].reshape(PQ, 512),
                         Afl[:, 16:32, :].reshape(PQ, 512)], axis=1)
    # bwd cols (contract over l): Ab[pq, 512g + 32*i' + l] = Ahat[pq, 16g+i', l]
    Ab = np.concatenate([Ahat[:, 0:16, :].reshape(PQ, 512),
                         Ahat[:, 16:32, :].reshape(PQ, 512)], axis=1)
    afb_full = np.concatenate([Af, Ab], axis=1)        # [289, 2048]

    cst = {}
    for c, (off, r) in enumerate(CHUNKS):
        cst[f"afb{c}"] = afb_full[off:off + r].astype(F16)
    state0 = np.zeros((128, 32), F32)
    for gp in range(2):
        state0[64 * gp:64 * gp + 32] = alpha
        state0[64 * gp + 32:64 * gp + 64] = Omega[:, 0]
    cst["state0"] = state0.astype(F32)
    sfin = np.zeros((64, 32), F32)
    for gp in range(2):
        for n in range(32):
            sfin[32 * gp + n, n] = 1.0
    cst["sfin"] = sfin.astype(F16)
    return cst


def kernel(actions, obss, Wa, ba, Wo, bo, alpha, A, Omega):
    actions = np.asarray(actions, F32)
    obss = np.asarray(obss, F32)
    Wa = np.asarray(Wa, F32); ba = np.asarray(ba, F32)
    Wo = np.asarray(Wo, F32); bo = np.asarray(bo, F32)
    alpha = np.asarray(alpha, F32)
    A = np.asarray(A, F32)
    Omega = np.asarray(Omega, F32)

    cst = _consts(Wa, ba, Wo, bo, alpha, A, Omega)
    in_maps = []
    for c in range(NCORES):
        reps = _prep_core(actions[NL * c:NL * c + NL],
                          obss[NL * c:NL * c + NL])
        in_maps.append({**reps, **cst})

    if "nc" not in _CACHE:
        _CACHE["nc"] = _build_bass()
    from concourse.bass_utils import run_bass_kernel_spmd
    r = run_bass_kernel_spmd(_CACHE["nc"], in_maps, list(range(NCORES)))
    outs = []
    for c in range(NCORES):
        o = r.results[c]["out"] if isinstance(r.results[c], dict) else r.results[c]
        outs.append(np.asarray(o, F32).reshape(NL))
    return np.concatenate(outs).astype(F32)
